# revision 26
# baseline (speedup 1.0000x reference)
"""Trainium2 Bass kernel for nn_Attention (B=2, S=2048, NX=1024, NH=16, HD=64).

Sharding: tensor-parallel over heads — each of 8 cores owns 2 heads.
Per core:
  phase 1: QT/KT/VT = W_slice^T @ X^T (fp32r matmuls, N=512), V transposed
           back to natural layout on the PE (for the EV matmul + `present`).
  phase 2: per (batch, head): ET = exp(K Q^T / 8) tiles on ACT;
           EV^T = [V | 1]^T @ ET accumulated in PSUM (row 64 = softmax sums);
           normalize with reciprocal + broadcast matmul.
  phase 3: AllToAll redistributes attention output so each core holds all
           1024 head-dims for its 512 sequence rows; local w_proj matmul.
Host: pure gather/transpose to assemble (out, present).
"""

import sys

sys.path.insert(0, "/opt/trn_rl_repo")

import numpy as np

import concourse.bass as bass  # noqa: F401  (registers AP machinery)
import concourse.mybir as mybir
import concourse.tile as tile
from concourse import bacc
from concourse.bass_utils import run_bass_kernel_spmd
from concourse.masks import make_identity

B, S, NX, NH, HD = 2, 2048, 1024, 16, 64
BS = B * S  # 4096
P = 128
KT8 = NX // P  # 8 contraction tiles
NB = BS // 512  # 8 column blocks
N_CORES = 8
F32 = mybir.dt.float32
F32R = mybir.dt.float32r
EXP = mybir.ActivationFunctionType.Exp

_prog_cache = {}


def r(ap):
    """fp32 AP -> fp32r view (full-rate PE, ~1e-4 rel err)."""
    return ap.bitcast(F32R)


def _build_program():
    nc = bacc.Bacc("TRN2", num_devices=N_CORES)

    xt_d = nc.declare_dram_parameter("xt", [NX, BS], F32, isOutput=False)
    wq_d = nc.declare_dram_parameter("wq", [NX, P], F32, isOutput=False)
    wk_d = nc.declare_dram_parameter("wk", [NX, P], F32, isOutput=False)
    wv_d = nc.declare_dram_parameter("wv", [NX, P], F32, isOutput=False)
    bq_d = nc.declare_dram_parameter("bq", [P], F32, isOutput=False)
    bk_d = nc.declare_dram_parameter("bk", [P], F32, isOutput=False)
    bv_d = nc.declare_dram_parameter("bv", [P], F32, isOutput=False)
    wp_d = nc.declare_dram_parameter("wp", [NX, NX], F32, isOutput=False)
    bp_d = nc.declare_dram_parameter("bp", [NX], F32, isOutput=False)

    kt_o = nc.declare_dram_parameter("kt", [P, BS], F32, isOutput=True)
    v_o = nc.declare_dram_parameter("v", [P, 32, 130], F32, isOutput=True)
    y_o = nc.declare_dram_parameter("y", [BS // N_CORES, NX], F32, isOutput=True)

    with tile.TileContext(nc) as tc, tc.tile_pool(name="statics", bufs=1) as statics:
        # ---------- static SBUF ----------
        wq_sb = statics.tile([P, KT8, P], F32R, tag="wq_sb", name="wq_sb")
        wk_sb = statics.tile([P, KT8, P], F32R, tag="wk_sb", name="wk_sb")
        wv_sb = statics.tile([P, KT8, P], F32R, tag="wv_sb", name="wv_sb")
        bq_sb = statics.tile([P, 1], F32, tag="bq_sb", name="bq_sb")
        bk_sb = statics.tile([P, 1], F32, tag="bk_sb", name="bk_sb")
        bv_sb = statics.tile([P, 1], F32, tag="bv_sb", name="bv_sb")
        ident = statics.tile([P, P], F32, tag="ident", name="ident")
        QT = statics.tile([P, BS], F32R, tag="QT", name="QT")
        KT = statics.tile([P, BS], F32R, tag="KT", name="KT")
        VN = statics.tile([P, 32, 130], F32R, tag="VN", name="VN")  # [s%128, s//128, 65*l + d], col 64/129 = 1.0
        OUT = statics.tile([P, BS], F32, tag="OUT", name="OUT")  # attention output^T (2 heads x 64 dims)
        WP = statics.tile([P, KT8, NX], F32R, tag="WP", name="WP")
        AGS = statics.tile([P, KT8, 512], F32R, tag="AGS", name="AGS")
        bp_b = statics.tile([P, NX], F32, tag="bp_b", name="bp_b")

        nc.sync.dma_start(wq_sb[:], r(wq_d.rearrange("(o p) m -> p o m", p=P)))
        nc.sync.dma_start(wk_sb[:], r(wk_d.rearrange("(o p) m -> p o m", p=P)))
        nc.sync.dma_start(wv_sb[:], r(wv_d.rearrange("(o p) m -> p o m", p=P)))
        nc.sync.dma_start(bq_sb[:], bq_d[:, None])
        nc.sync.dma_start(bk_sb[:], bk_d[:, None])
        nc.sync.dma_start(bv_sb[:], bv_d[:, None])
        make_identity(nc, ident[:])
        # memset can't target fp32r; init the VN "ones" columns via DVE copies
        scr1 = statics.tile([P, 1], F32, tag="scr1", name="scr1")
        nc.vector.memset(scr1[:], 1.0)
        nc.vector.tensor_copy(VN[:, :, 64], scr1[:].to_broadcast((P, 32)))
        nc.vector.tensor_copy(VN[:, :, 129], scr1[:].to_broadcast((P, 32)))

        # ---------- phase 1: QT / KT / V ----------
        with (
            tc.tile_pool(name="xtp", bufs=6) as xtp,
            tc.tile_pool(name="vtp", bufs=2) as vtp,
            tc.tile_pool(name="ph1ps", bufs=2, space="PSUM") as ph1ps,
        ):
            for n in range(NB):
                pq = ph1ps.tile([P, 512], F32, tag="q", name="pq")
                pk = ph1ps.tile([P, 512], F32, tag="k", name="pk")
                pv = ph1ps.tile([P, 512], F32, tag="v", name="pv")
                for k in range(KT8):
                    xt_t = xtp.tile([P, 512], F32R, tag="xt", name="xt_t")
                    nc.sync.dma_start(
                        xt_t[:], r(xt_d[k * P : (k + 1) * P, n * 512 : (n + 1) * 512])
                    )
                    st = dict(start=(k == 0), stop=(k == KT8 - 1))
                    nc.tensor.matmul(pq[:], r(wq_sb[:, k]), r(xt_t[:]), **st)
                    nc.tensor.matmul(pk[:], r(wk_sb[:, k]), r(xt_t[:]), **st)
                    nc.tensor.matmul(pv[:], r(wv_sb[:, k]), r(xt_t[:]), **st)
                sl = slice(n * 512, (n + 1) * 512)
                nc.vector.tensor_scalar_add(QT[:, sl], pq[:], bq_sb[:])
                nc.vector.tensor_scalar_add(KT[:, sl], pk[:], bk_sb[:])
                vt_sb = vtp.tile([P, 512], F32, tag="vt", name="vt_sb")
                nc.vector.tensor_scalar_add(vt_sb[:], pv[:], bv_sb[:])
                for c4 in range(4):
                    pt = ph1ps.tile([P, P], F32, tag="pt", name="pt")
                    nc.tensor.transpose(pt[:], vt_sb[:, c4 * P : (c4 + 1) * P], ident[:])
                    ch = 4 * n + c4
                    nc.vector.tensor_copy(VN[:, ch, 0:64], pt[:, 0:64])
                    nc.vector.tensor_copy(VN[:, ch, 65:129], pt[:, 64:128])

        # `present` outputs + preloads for later phases (overlap with attention)
        nc.sync.dma_start(kt_o[:], KT[:].bitcast(F32))
        nc.sync.dma_start(v_o[:], VN[:].bitcast(F32))
        nc.sync.dma_start(WP[:], r(wp_d.rearrange("(o p) m -> p o m", p=P)))

        # ---------- phase 2: attention ----------
        # Per (batch, 512-wide query block). Per key chunk sk: the two heads'
        # K=64 score matmuls sit on PE row-groups 0-63 / 64-127 and are emitted
        # adjacently so they execute concurrently; one exp covers both heads.
        # PSUM: et 2x[128,1024] + ev{0,1} 2x[128,512] = 8 banks, all 2-buffered.
        _dram_cm = tc.tile_pool(name="dram", bufs=2, space="DRAM")
        dramp = _dram_cm.__enter__()
        ag_in = dramp.tile([N_CORES, P, 512], F32, name="ag_in")
        ag_out = dramp.tile([N_CORES, P, 512], F32, name="ag_out")
        with (
            tc.tile_pool(name="etsbp", bufs=4) as etsbp,
            tc.tile_pool(name="rbp", bufs=2) as rbp,
            tc.tile_pool(name="etps", bufs=3, space="PSUM") as etps,
            tc.tile_pool(name="evps", bufs=1, space="PSUM") as evps,
        ):
            for b in range(B):
                for qb in range(4):
                    col0 = 2048 * b + 512 * qb
                    evs = [
                        evps.tile([P, 512], F32, tag=f"ev{l}", name=f"ev{l}")
                        for l in range(2)
                    ]
                    # software-pipelined: EV(sk-1) is emitted after scores(sk)/exp(sk)
                    # so the PE never head-of-line blocks on the exp it just fed.
                    pend = None
                    for sk in range(17):
                        if sk < 16:
                            ksl = slice(2048 * b + 128 * sk, 2048 * b + 128 * sk + 128)
                            et_ps = etps.tile([P, 1024], F32, tag="et", name="et_ps")
                            for l in range(2):
                                hsl = slice(64 * l, 64 * l + 64)
                                nc.tensor.matmul(
                                    et_ps[:, 512 * l : 512 * l + 512],
                                    r(KT[hsl, ksl]),
                                    r(QT[hsl, col0 : col0 + 512]),
                                    start=True,
                                    stop=True,
                                )
                            et_sb = etsbp.tile([P, 1024], F32R, tag="etsb", name="et_sb")
                            nc.scalar.activation(et_sb[:], et_ps[:], EXP, scale=0.125)
                        if pend is not None:
                            pj, p_sb = pend
                            for l in range(2):
                                nc.tensor.matmul(
                                    evs[l][0:65, :],
                                    r(VN[:, 16 * b + pj, 65 * l : 65 * l + 65]),
                                    r(p_sb[:, 512 * l : 512 * l + 512]),
                                    start=(pj == 0),
                                    stop=(pj == 15),
                                    skip_group_check=True,
                                )
                        pend = (sk, et_sb) if sk < 16 else None
                    # softmax normalization: out = ev[0:64] * (1 / ev[64])
                    for l in range(2):
                        sums_sb = rbp.tile([1, 512], F32, tag="sums", name="sums_sb")
                        nc.vector.tensor_copy(sums_sb[:], evs[l][64:65, :])
                        rd0 = dramp.tile([1, 512], F32, tag="rd0", name="rd0")
                        nc.sync.dma_start(rd0[:], sums_sb[:])
                        s128 = rbp.tile([P, 4], F32, tag="s128", name="s128")
                        nc.sync.dma_start(
                            s128[:], rd0.rearrange("o (p f) -> (o p) f", p=P)
                        )
                        ri = rbp.tile([P, 4], F32, tag="ri", name="ri")
                        nc.vector.reciprocal(ri[:], s128[:])
                        rd = dramp.tile([P, 4], F32, tag="rd", name="rd")
                        nc.sync.dma_start(rd[:], ri[:])
                        rb_sb = rbp.tile([64, 512], F32, tag="rb", name="rb_sb")
                        nc.sync.dma_start(
                            rb_sb[:],
                            rd.rearrange("p f -> (p f)")[None, :].to_broadcast((64, 512)),
                        )
                        nc.vector.tensor_mul(
                            OUT[64 * l : 64 * l + 64, col0 : col0 + 512],
                            evs[l][0:64, :],
                            rb_sb[:],
                        )
                    nc.sync.dma_start(
                        ag_in[4 * b + qb], OUT[:, col0 : col0 + 512]
                    )

        # ---------- phase 3: AllToAll + projection ----------
        if True:
            nc.gpsimd.collective_compute(
                "AllToAll",
                mybir.AluOpType.bypass,
                replica_groups=[list(range(N_CORES))],
                ins=[ag_in.opt()],
                outs=[ag_out.opt()],
            )
            nc.sync.dma_start(AGS[:], r(ag_out.rearrange("o p m -> p o m")))

            with (
                tc.tile_pool(name="ysbp", bufs=2) as ysbp,
                tc.tile_pool(name="pps", bufs=2, space="PSUM") as pps,
            ):
                nc.sync.dma_start(bp_b[0:1, :], bp_d[None, :])
                # DEBUG: bias add disabled below

                for m in range(4):
                    yp = pps.tile([P, NX], F32, tag="y", name="yp")
                    for k in range(KT8):
                        st = dict(start=(k == 0), stop=(k == KT8 - 1))
                        lh = AGS[:, k, m * P : (m + 1) * P]
                        nc.tensor.matmul(yp[:, 0:512], r(lh), r(WP[:, k, 0:512]), **st)
                        nc.tensor.matmul(yp[:, 512:1024], r(lh), r(WP[:, k, 512:1024]), **st)
                    y_sb = ysbp.tile([P, NX], F32, tag="ysb", name="y_sb")
                    nc.vector.tensor_copy(y_sb[:], yp[:])
                    nc.sync.dma_start(y_o[m * P : (m + 1) * P, :], y_sb[:])

    nc.finalize()
    return nc


def _run(inputs, trace=False, trace_kwargs=None):
    x = np.asarray(inputs["x"], dtype=np.float32)
    w_attn = np.asarray(inputs["w_attn"], dtype=np.float32)
    b_attn = np.asarray(inputs["b_attn"], dtype=np.float32)
    w_proj = np.asarray(inputs["w_proj"], dtype=np.float32)
    b_proj = np.asarray(inputs["b_proj"], dtype=np.float32)

    if "nc" not in _prog_cache:
        _prog_cache["nc"] = _build_program()
    nc = _prog_cache["nc"]

    xt = np.ascontiguousarray(x.reshape(BS, NX).T)
    wp = np.ascontiguousarray(w_proj)
    bp = np.ascontiguousarray(b_proj)
    in_maps = []
    for c in range(N_CORES):
        sl = slice(P * c, P * (c + 1))
        in_maps.append(
            {
                "xt": xt,
                "wq": np.ascontiguousarray(w_attn[:, sl]),
                "wk": np.ascontiguousarray(w_attn[:, NX:][:, sl]),
                "wv": np.ascontiguousarray(w_attn[:, 2 * NX:][:, sl]),
                "bq": np.ascontiguousarray(b_attn[sl]),
                "bk": np.ascontiguousarray(b_attn[NX:][sl]),
                "bv": np.ascontiguousarray(b_attn[2 * NX:][sl]),
                "wp": wp,
                "bp": bp,
            }
        )

    res = run_bass_kernel_spmd(
        nc, in_maps, list(range(N_CORES)), trace=trace, **(trace_kwargs or {})
    )

    # ---- host-side gather / unshard ----
    out = np.concatenate([res.results[c]["y"] for c in range(N_CORES)], axis=0)
    out = out.reshape(B, S, NX)

    k_full = np.empty((B, NH, S, HD), dtype=np.float32)
    v_full = np.empty((B, NH, S, HD), dtype=np.float32)
    for c in range(N_CORES):
        kt = res.results[c]["kt"]  # [128, BS]
        vr = res.results[c]["v"]  # [128, 32, 130]
        for l in range(2):
            h = 2 * c + l
            k_full[:, h] = kt[64 * l : 64 * l + 64].reshape(HD, B, S).transpose(1, 2, 0)
            # vr[p, 16*b + sc, 65*l + d] = v[b, 128*sc + p, d]
            vv = vr[:, :, 65 * l : 65 * l + 64]  # [128, 32, 64]
            v_full[:, h] = vv.transpose(1, 0, 2).reshape(B, S, HD)
    present = np.stack([k_full, v_full])

    if trace:
        return (out, present), res
    return out, present


def kernel(**inputs):
    return _run(inputs, trace=False)


# revision 29
# speedup vs baseline: 1.2534x; 1.2534x over previous
"""Trainium2 Bass kernel for nn_Attention (B=2, S=2048, NX=1024, NH=16, HD=64).

Sharding: tensor-parallel over heads — each of 8 cores owns 2 heads.
Per core:
  phase 1: QT/KT/VT = W_slice^T @ X^T (fp32r matmuls, N=512), V transposed
           back to natural layout on the PE (for the EV matmul + `present`).
  phase 2: per (batch, head): ET = exp(K Q^T / 8) tiles on ACT;
           EV^T = [V | 1]^T @ ET accumulated in PSUM (row 64 = softmax sums);
           normalize with reciprocal + broadcast matmul.
  phase 3: AllToAll redistributes attention output so each core holds all
           1024 head-dims for its 512 sequence rows; local w_proj matmul.
Host: pure gather/transpose to assemble (out, present).
"""

import sys

sys.path.insert(0, "/opt/trn_rl_repo")

import numpy as np

import concourse.bass as bass  # noqa: F401  (registers AP machinery)
import concourse.mybir as mybir
import concourse.tile as tile
from concourse import bacc
from concourse.bass_utils import run_bass_kernel_spmd
from concourse.masks import make_identity

B, S, NX, NH, HD = 2, 2048, 1024, 16, 64
BS = B * S  # 4096
P = 128
KT8 = NX // P  # 8 contraction tiles
NB = BS // 512  # 8 column blocks
N_CORES = 8
F32 = mybir.dt.float32
F32R = mybir.dt.float32r
EXP = mybir.ActivationFunctionType.Exp

_prog_cache = {}


def r(ap):
    """fp32 AP -> fp32r view (full-rate PE, ~1e-4 rel err)."""
    return ap.bitcast(F32R)


def _build_program():
    nc = bacc.Bacc("TRN2", num_devices=N_CORES)

    xt_d = nc.declare_dram_parameter("xt", [NX, BS], F32, isOutput=False)
    wq_d = nc.declare_dram_parameter("wq", [NX, P], F32, isOutput=False)
    wk_d = nc.declare_dram_parameter("wk", [NX, P], F32, isOutput=False)
    wv_d = nc.declare_dram_parameter("wv", [NX, P], F32, isOutput=False)
    bq_d = nc.declare_dram_parameter("bq", [P], F32, isOutput=False)
    bk_d = nc.declare_dram_parameter("bk", [P], F32, isOutput=False)
    bv_d = nc.declare_dram_parameter("bv", [P], F32, isOutput=False)
    wp_d = nc.declare_dram_parameter("wp", [NX, NX], F32, isOutput=False)
    bp_d = nc.declare_dram_parameter("bp", [NX], F32, isOutput=False)

    kt_o = nc.declare_dram_parameter("kt", [P, BS], F32, isOutput=True)
    v_o = nc.declare_dram_parameter("v", [P, 32, 130], F32, isOutput=True)
    y_o = nc.declare_dram_parameter("y", [BS // N_CORES, NX], F32, isOutput=True)

    with tile.TileContext(nc) as tc, tc.tile_pool(name="statics", bufs=1) as statics:
        # ---------- static SBUF ----------
        wq_sb = statics.tile([P, KT8, P], F32R, tag="wq_sb", name="wq_sb")
        wk_sb = statics.tile([P, KT8, P], F32R, tag="wk_sb", name="wk_sb")
        wv_sb = statics.tile([P, KT8, P], F32R, tag="wv_sb", name="wv_sb")
        bq_sb = statics.tile([P, 1], F32, tag="bq_sb", name="bq_sb")
        bk_sb = statics.tile([P, 1], F32, tag="bk_sb", name="bk_sb")
        bv_sb = statics.tile([P, 1], F32, tag="bv_sb", name="bv_sb")
        ident = statics.tile([P, P], F32, tag="ident", name="ident")
        QT = statics.tile([P, BS], F32R, tag="QT", name="QT")
        KT = statics.tile([P, BS], F32R, tag="KT", name="KT")
        VN = statics.tile([P, 32, 130], F32R, tag="VN", name="VN")  # [s%128, s//128, 65*l + d], col 64/129 = 1.0
        OUT = statics.tile([P, BS], F32, tag="OUT", name="OUT")  # attention output^T (2 heads x 64 dims)
        WP = statics.tile([P, KT8, NX], F32R, tag="WP", name="WP")
        AGS0 = statics.tile([P, KT8, 256], F32R, tag="AGS0", name="AGS0")
        AGS1 = statics.tile([P, KT8, 256], F32R, tag="AGS1", name="AGS1")
        bp_b = statics.tile([P, NX], F32, tag="bp_b", name="bp_b")

        nc.sync.dma_start(wq_sb[:], r(wq_d.rearrange("(o p) m -> p o m", p=P)))
        nc.sync.dma_start(wk_sb[:], r(wk_d.rearrange("(o p) m -> p o m", p=P)))
        nc.sync.dma_start(wv_sb[:], r(wv_d.rearrange("(o p) m -> p o m", p=P)))
        nc.sync.dma_start(bq_sb[:], bq_d[:, None])
        nc.sync.dma_start(bk_sb[:], bk_d[:, None])
        nc.sync.dma_start(bv_sb[:], bv_d[:, None])
        make_identity(nc, ident[:])
        # memset can't target fp32r; init the VN "ones" columns via DVE copies
        scr1 = statics.tile([P, 1], F32, tag="scr1", name="scr1")
        nc.vector.memset(scr1[:], 1.0)
        nc.vector.tensor_copy(VN[:, :, 64], scr1[:].to_broadcast((P, 32)))
        nc.vector.tensor_copy(VN[:, :, 129], scr1[:].to_broadcast((P, 32)))

        # ---------- phase 1: QT / KT / V ----------
        with (
            tc.tile_pool(name="xtp", bufs=6) as xtp,
            tc.tile_pool(name="vtp", bufs=2) as vtp,
            tc.tile_pool(name="ph1ps", bufs=2, space="PSUM") as ph1ps,
        ):
            for n in range(NB):
                pq = ph1ps.tile([P, 512], F32, tag="q", name="pq")
                pk = ph1ps.tile([P, 512], F32, tag="k", name="pk")
                pv = ph1ps.tile([P, 512], F32, tag="v", name="pv")
                for k in range(KT8):
                    xt_t = xtp.tile([P, 512], F32R, tag="xt", name="xt_t")
                    nc.sync.dma_start(
                        xt_t[:], r(xt_d[k * P : (k + 1) * P, n * 512 : (n + 1) * 512])
                    )
                    st = dict(start=(k == 0), stop=(k == KT8 - 1))
                    nc.tensor.matmul(pq[:], r(wq_sb[:, k]), r(xt_t[:]), **st)
                    nc.tensor.matmul(pk[:], r(wk_sb[:, k]), r(xt_t[:]), **st)
                    nc.tensor.matmul(pv[:], r(wv_sb[:, k]), r(xt_t[:]), **st)
                sl = slice(n * 512, (n + 1) * 512)
                nc.vector.tensor_scalar_add(QT[:, sl], pq[:], bq_sb[:])
                nc.vector.tensor_scalar_add(KT[:, sl], pk[:], bk_sb[:])
                vt_sb = vtp.tile([P, 512], F32, tag="vt", name="vt_sb")
                nc.vector.tensor_scalar_add(vt_sb[:], pv[:], bv_sb[:])
                for c4 in range(4):
                    pt = ph1ps.tile([P, P], F32, tag="pt", name="pt")
                    nc.tensor.transpose(pt[:], vt_sb[:, c4 * P : (c4 + 1) * P], ident[:])
                    ch = 4 * n + c4
                    nc.vector.tensor_copy(VN[:, ch, 0:64], pt[:, 0:64])
                    nc.vector.tensor_copy(VN[:, ch, 65:129], pt[:, 64:128])

        # `present` outputs + preloads for later phases (overlap with attention)
        nc.sync.dma_start(kt_o[:], KT[:].bitcast(F32))
        nc.sync.dma_start(v_o[:], VN[:].bitcast(F32))
        nc.sync.dma_start(WP[:], r(wp_d.rearrange("(o p) m -> p o m", p=P)))

        # ---------- phase 2: attention ----------
        # Per (batch, 512-wide query block). Per key chunk sk: the two heads'
        # K=64 score matmuls sit on PE row-groups 0-63 / 64-127 and are emitted
        # adjacently so they execute concurrently; one exp covers both heads.
        # PSUM: et 2x[128,1024] + ev{0,1} 2x[128,512] = 8 banks, all 2-buffered.
        _dram_cm = tc.tile_pool(name="dram", bufs=2, space="DRAM")
        dramp = _dram_cm.__enter__()
        ag_in0 = dramp.tile([N_CORES, P, 256], F32, name="ag_in0")
        ag_out0 = dramp.tile([N_CORES, P, 256], F32, name="ag_out0")
        ag_in1 = dramp.tile([N_CORES, P, 256], F32, name="ag_in1")
        ag_out1 = dramp.tile([N_CORES, P, 256], F32, name="ag_out1")
        with (
            tc.tile_pool(name="etsbp", bufs=4) as etsbp,
            tc.tile_pool(name="rbp", bufs=2) as rbp,
            tc.tile_pool(name="etps", bufs=2, space="PSUM") as etps,
            tc.tile_pool(name="evps", bufs=2, space="PSUM") as evps,
        ):
            for b in range(B):
                if b == 1:
                    # batch-0 A2A + AGS load overlap batch-1 attention
                    nc.gpsimd.collective_compute(
                        "AllToAll",
                        mybir.AluOpType.bypass,
                        replica_groups=[list(range(N_CORES))],
                        ins=[ag_in0.opt()],
                        outs=[ag_out0.opt()],
                    )
                    nc.sync.dma_start(AGS0[:], r(ag_out0.rearrange("o p m -> p o m")))
                for qb in range(4):
                    col0 = 2048 * b + 512 * qb
                    evs = [
                        evps.tile([P, 512], F32, tag=f"ev{l}", name=f"ev{l}")
                        for l in range(2)
                    ]
                    # software-pipelined: EV(sk-1) is emitted after scores(sk)/exp(sk)
                    # so the PE never head-of-line blocks on the exp it just fed.
                    pend = None
                    for sk in range(17):
                        if sk < 16:
                            ksl = slice(2048 * b + 128 * sk, 2048 * b + 128 * sk + 128)
                            et_ps = etps.tile([P, 1024], F32, tag="et", name="et_ps")
                            for l in range(2):
                                hsl = slice(64 * l, 64 * l + 64)
                                nc.tensor.matmul(
                                    et_ps[:, 512 * l : 512 * l + 512],
                                    r(KT[hsl, ksl]),
                                    r(QT[hsl, col0 : col0 + 512]),
                                    start=True,
                                    stop=True,
                                )
                            et_sb = etsbp.tile([P, 1024], F32R, tag="etsb", name="et_sb")
                            nc.scalar.activation(et_sb[:], et_ps[:], EXP, scale=0.125)
                        if pend is not None:
                            pj, p_sb = pend
                            for l in range(2):
                                nc.tensor.matmul(
                                    evs[l][0:65, :],
                                    r(VN[:, 16 * b + pj, 65 * l : 65 * l + 65]),
                                    r(p_sb[:, 512 * l : 512 * l + 512]),
                                    start=(pj == 0),
                                    stop=(pj == 15),
                                    skip_group_check=True,
                                )
                        pend = (sk, et_sb) if sk < 16 else None
                    # softmax normalization: out = ev[0:64] * (1 / ev[64])
                    for l in range(2):
                        sums_sb = rbp.tile([1, 512], F32, tag="sums", name="sums_sb")
                        nc.vector.tensor_copy(sums_sb[:], evs[l][64:65, :])
                        rd0 = dramp.tile([1, 512], F32, tag="rd0", name="rd0")
                        nc.sync.dma_start(rd0[:], sums_sb[:])
                        s128 = rbp.tile([P, 4], F32, tag="s128", name="s128")
                        nc.sync.dma_start(
                            s128[:], rd0.rearrange("o (p f) -> (o p) f", p=P)
                        )
                        ri = rbp.tile([P, 4], F32, tag="ri", name="ri")
                        nc.vector.reciprocal(ri[:], s128[:])
                        rd = dramp.tile([P, 4], F32, tag="rd", name="rd")
                        nc.sync.dma_start(rd[:], ri[:])
                        rb_sb = rbp.tile([64, 512], F32, tag="rb", name="rb_sb")
                        nc.sync.dma_start(
                            rb_sb[:],
                            rd.rearrange("p f -> (p f)")[None, :].to_broadcast((64, 512)),
                        )
                        nc.vector.tensor_mul(
                            OUT[64 * l : 64 * l + 64, col0 : col0 + 512],
                            evs[l][0:64, :],
                            rb_sb[:],
                        )
                    agi = ag_in0 if b == 0 else ag_in1
                    nc.sync.dma_start(
                        agi[2 * qb : 2 * qb + 2].rearrange("j p m -> p j m"),
                        OUT[:, col0 : col0 + 512].rearrange("p (j m) -> p j m", j=2),
                    )

        # ---------- phase 3: AllToAll + projection ----------
        if True:
            nc.gpsimd.collective_compute(
                "AllToAll",
                mybir.AluOpType.bypass,
                replica_groups=[list(range(N_CORES))],
                ins=[ag_in1.opt()],
                outs=[ag_out1.opt()],
            )
            nc.sync.dma_start(AGS1[:], r(ag_out1.rearrange("o p m -> p o m")))

            with (
                tc.tile_pool(name="ysbp", bufs=2) as ysbp,
                tc.tile_pool(name="pps", bufs=2, space="PSUM") as pps,
            ):
                nc.sync.dma_start(bp_b[:], bp_d[None, :].to_broadcast((P, NX)))
                for b2 in range(2):
                    ags = AGS0 if b2 == 0 else AGS1
                    for m in range(2):
                        yp = pps.tile([P, NX], F32, tag="y", name="yp")
                        for k in range(KT8):
                            st = dict(start=(k == 0), stop=(k == KT8 - 1))
                            lh = ags[:, k, m * P : (m + 1) * P]
                            nc.tensor.matmul(yp[:, 0:512], r(lh), r(WP[:, k, 0:512]), **st)
                            nc.tensor.matmul(yp[:, 512:1024], r(lh), r(WP[:, k, 512:1024]), **st)
                        y_sb = ysbp.tile([P, NX], F32, tag="ysb", name="y_sb")
                        nc.vector.tensor_add(y_sb[:], yp[:], bp_b[:])
                        nc.sync.dma_start(
                            y_o[256 * b2 + m * P : 256 * b2 + (m + 1) * P, :], y_sb[:]
                        )

    nc.finalize()
    return nc


def _run(inputs, trace=False, trace_kwargs=None):
    x = np.asarray(inputs["x"], dtype=np.float32)
    w_attn = np.asarray(inputs["w_attn"], dtype=np.float32)
    b_attn = np.asarray(inputs["b_attn"], dtype=np.float32)
    w_proj = np.asarray(inputs["w_proj"], dtype=np.float32)
    b_proj = np.asarray(inputs["b_proj"], dtype=np.float32)

    if "nc" not in _prog_cache:
        _prog_cache["nc"] = _build_program()
    nc = _prog_cache["nc"]

    xt = np.ascontiguousarray(x.reshape(BS, NX).T)
    wp = np.ascontiguousarray(w_proj)
    bp = np.ascontiguousarray(b_proj)
    in_maps = []
    for c in range(N_CORES):
        sl = slice(P * c, P * (c + 1))
        in_maps.append(
            {
                "xt": xt,
                "wq": np.ascontiguousarray(w_attn[:, sl]),
                "wk": np.ascontiguousarray(w_attn[:, NX:][:, sl]),
                "wv": np.ascontiguousarray(w_attn[:, 2 * NX:][:, sl]),
                "bq": np.ascontiguousarray(b_attn[sl]),
                "bk": np.ascontiguousarray(b_attn[NX:][sl]),
                "bv": np.ascontiguousarray(b_attn[2 * NX:][sl]),
                "wp": wp,
                "bp": bp,
            }
        )

    res = run_bass_kernel_spmd(
        nc, in_maps, list(range(N_CORES)), trace=trace, **(trace_kwargs or {})
    )

    # ---- host-side gather / unshard ----
    out = np.empty((B, S, NX), dtype=np.float32)
    for c in range(N_CORES):
        yv = res.results[c]["y"]
        out[0, 256 * c : 256 * c + 256] = yv[0:256]
        out[1, 256 * c : 256 * c + 256] = yv[256:512]

    k_full = np.empty((B, NH, S, HD), dtype=np.float32)
    v_full = np.empty((B, NH, S, HD), dtype=np.float32)
    for c in range(N_CORES):
        kt = res.results[c]["kt"]  # [128, BS]
        vr = res.results[c]["v"]  # [128, 32, 130]
        for l in range(2):
            h = 2 * c + l
            k_full[:, h] = kt[64 * l : 64 * l + 64].reshape(HD, B, S).transpose(1, 2, 0)
            # vr[p, 16*b + sc, 65*l + d] = v[b, 128*sc + p, d]
            vv = vr[:, :, 65 * l : 65 * l + 64]  # [128, 32, 64]
            v_full[:, h] = vv.transpose(1, 0, 2).reshape(B, S, HD)
    present = np.stack([k_full, v_full])

    if trace:
        return (out, present), res
    return out, present


def kernel(**inputs):
    return _run(inputs, trace=False)
